# revision 2
# baseline (speedup 1.0000x reference)
import numpy as np
import jax
import jax.numpy as jnp
from functools import partial

# nn_CrossAttention: B=8 images sharded 1-per-NeuronCore (pure data parallel
# over batch; all convs and windowed attention are batch-independent).
DIM = 192
NUM_HEADS = 6
WS_M, WS_S = 8, 7
H = W = 112  # 112 % 8 == 0 and 112 % 7 == 0 -> pad/crop are no-ops


def _rel_positions(ws: int) -> np.ndarray:
    coords = np.stack(np.meshgrid(np.arange(ws), np.arange(ws), indexing='ij'))
    cf = coords.reshape(2, -1)
    rel = cf[:, :, None] - cf[:, None, :]
    rel = rel.transpose(1, 2, 0).astype(np.float32)
    return np.sign(rel) * np.log1p(np.abs(rel))


_RP_M = _rel_positions(WS_M)   # [64, 64, 2]
_RP_S = _rel_positions(WS_S)   # [49, 49, 2]


def _conv1x1(x, w, b):
    # x: [C,H,W]
    return jnp.einsum('chw,oc->ohw', x, w) + b[:, None, None]


def _dwconv5(x, w, b):
    xp = jnp.pad(x[None], ((0, 0), (0, 0), (2, 2), (2, 2)), mode='reflect')
    y = jax.lax.conv_general_dilated(
        xp, w, window_strides=(1, 1), padding='VALID',
        dimension_numbers=('NCHW', 'OIHW', 'NCHW'),
        feature_group_count=x.shape[0])
    return y[0] + b[:, None, None]


def _win_part(x, ws):
    # x: [H,W,C] -> [nW, ws*ws, C]
    Hx, Wx, C = x.shape
    x = x.reshape(Hx // ws, ws, Wx // ws, ws, C).transpose(0, 2, 1, 3, 4)
    return x.reshape(-1, ws * ws, C)


def _win_rev(win, ws, Hx, Wx):
    C = win.shape[-1]
    x = win.reshape(Hx // ws, Wx // ws, ws, ws, C).transpose(0, 2, 1, 3, 4)
    return x.reshape(Hx, Wx, C)


def _win_attn(qkv, rp, w1, b1, w2, b2):
    # qkv: [nW, N, 3*dim]
    B_, N, C3 = qkv.shape
    dim = C3 // 3
    hd = dim // NUM_HEADS
    qkv = qkv.reshape(B_, N, 3, NUM_HEADS, hd).transpose(2, 0, 3, 1, 4)
    q, k, v = qkv[0] * (hd ** -0.5), qkv[1], qkv[2]
    attn = jnp.einsum('bhnd,bhmd->bhnm', q, k)
    bias = jax.nn.relu(rp @ w1.T + b1) @ w2.T + b2        # [N,N,nh]
    attn = jax.nn.softmax(attn + bias.transpose(2, 0, 1)[None], axis=-1)
    out = jnp.einsum('bhnm,bhmd->bhnd', attn, v)
    return out.transpose(0, 2, 1, 3).reshape(B_, N, dim)


def _one_image(X, Y, Vm_w, Vm_b, Vs_w, Vs_b, QKm_w, QKm_b, QKs_w, QKs_b,
               convm_w, convm_b, convs_w, convs_b, proj_w, proj_b,
               mm_w1, mm_b1, mm_w2, mm_b2, ms_w1, ms_b1, ms_w2, ms_b2,
               rp_m, rp_s):
    V_m = _conv1x1(X, Vm_w, Vm_b)
    V_s = _conv1x1(Y, Vs_w, Vs_b)
    QK_m = _conv1x1(X, QKm_w, QKm_b)
    QK_s = _conv1x1(Y, QKs_w, QKs_b)
    qkv_m = jnp.concatenate([QK_m, V_s], axis=0)   # [576,H,W]
    qkv_s = jnp.concatenate([QK_s, V_m], axis=0)

    win_m = _win_part(qkv_m.transpose(1, 2, 0), WS_M)
    aw_m = _win_attn(win_m, rp_m, mm_w1, mm_b1, mm_w2, mm_b2)
    out_m = _win_rev(aw_m, WS_M, H, W)             # [H,W,C]

    win_s = _win_part(qkv_s.transpose(1, 2, 0), WS_S)
    aw_s = _win_attn(win_s, rp_s, ms_w1, ms_b1, ms_w2, ms_b2)
    out_s = _win_rev(aw_s, WS_S, H, W)

    attn_m = out_m.transpose(2, 0, 1)
    attn_s = out_s.transpose(2, 0, 1)
    conv_m = _dwconv5(V_m, convm_w, convm_b)
    conv_s = _dwconv5(V_s, convs_w, convs_b)
    main = _conv1x1(conv_m + attn_m, proj_w, proj_b)
    structure = _conv1x1(conv_s + attn_s, proj_w, proj_b)
    return main, structure


_WNAMES = ['Vm_w', 'Vm_b', 'Vs_w', 'Vs_b', 'QKm_w', 'QKm_b', 'QKs_w', 'QKs_b',
           'convm_w', 'convm_b', 'convs_w', 'convs_b', 'proj_w', 'proj_b',
           'mm_w1', 'mm_b1', 'mm_w2', 'mm_b2', 'ms_w1', 'ms_b1', 'ms_w2', 'ms_b2']

_pmapped = None


def _get_pmapped():
    global _pmapped
    if _pmapped is None:
        in_axes = (0, 0) + (None,) * (len(_WNAMES) + 2)
        _pmapped = jax.pmap(_one_image, in_axes=in_axes)
    return _pmapped


def kernel(**inputs):
    X = np.asarray(inputs['X'], dtype=np.float32)
    Y = np.asarray(inputs['Y'], dtype=np.float32)
    ws = [np.asarray(inputs[n], dtype=np.float32) for n in _WNAMES]
    try:
        if len(jax.devices()) < 8:
            raise RuntimeError("need 8 cores for pmap path")
        fn = _get_pmapped()
        main, structure = fn(X, Y, *ws, _RP_M, _RP_S)
    except Exception:
        # Fallback: run images sequentially on the default device.
        jit_one = jax.jit(_one_image)
        outs = [jit_one(X[b], Y[b], *ws, _RP_M, _RP_S) for b in range(X.shape[0])]
        main = jnp.stack([o[0] for o in outs])
        structure = jnp.stack([o[1] for o in outs])
    return (np.asarray(main, dtype=np.float32),
            np.asarray(structure, dtype=np.float32))
